# revision 28
# baseline (speedup 1.0000x reference)
"""DeepseekV3 MLA attention kernel for 8 Trainium2 NeuronCores.

Sharding: 2-way data-parallel over batch x 4-way tensor-parallel over heads.
Core c handles batch b = c // 4 and heads [4*(c%4) .. 4*(c%4)+4).

All matmul operands are bf16 (psum accumulation stays f32); intermediates
(qT, kv latent, kT, v) are SBUF-resident for the whole kernel -- no DRAM
scratch round-trips. Weights and tables load once and are shared across
timing reps. Per s-block the q/kv projections (phase 1) and the k/v head
projections (phase 2) are interleaved so the PE never drains. Attention
(phase 3) runs the two heads of a rope-pair interleaved through the
k-loop (pipeline depth), with the scoresT layout and max-free softmax:
exp on the scalar engine, causal mask + denominator accumulation on the
vector engine, the partition reduction via gpsimd partition_all_reduce,
and 1/denominator scaling on the vector engine. Diagonal k-tiles shrink
their moving dim to the surviving causal q-range. RoPE rotate_half is a
128x128 permutation matmul on the PE. The w_o stage for q-tile t is
emitted after the attention of q-tile t+1 so its softmax tail chains
hide under PE work. Host sums the 4 partial outputs per batch.
"""

from contextlib import ExitStack
from dataclasses import dataclass

import numpy as np

import concourse.bacc as bacc
import concourse.mybir as mybir
import concourse.tile as tile
from concourse import bass_isa

F32 = mybir.dt.float32
F32R = mybir.dt.float32r
BF16 = mybir.dt.bfloat16


@dataclass(frozen=True)
class Cfg:
    S: int = 2048          # sequence length (per batch)
    HID: int = 2048        # hidden dim
    QLR: int = 1536        # q lora rank (host-side only)
    KVLR: int = 512        # kv lora rank
    NH_G: int = 4          # heads per core
    DN: int = 128          # nope dim
    DR: int = 64           # rope dim
    DV: int = 128          # v head dim
    ST: int = 512          # phase-1/2 s-block width
    QT: int = 512          # attention q-tile width

    @property
    def QFN(self):
        return self.NH_G * self.DN      # fused q nope cols

    @property
    def QFR(self):
        return (self.NH_G // 2) * 128   # fused q rope cols (pair-packed)

    @property
    def SCALE(self):
        return 1.0 / float(np.sqrt(self.DN + self.DR))


CFG = Cfg()


def build_nc(C: Cfg, reps: int = 1):
    nc = bacc.Bacc("TRN2", target_bir_lowering=False, debug=False, num_devices=8)
    P = 128
    HO = C.HID // P
    NS = C.S // C.ST
    KVC = C.KVLR // P
    NPAIR = C.NH_G // 2
    NQT = C.S // C.QT
    NDIAG = C.QT // P
    NVS = C.S // P
    NQN = C.QFN // P
    NOT = C.HID // 512

    # ---- kernel I/O (bf16 unless noted) ----
    hT = nc.dram_tensor("hT", [C.HID, C.S], BF16, kind="ExternalInput").ap()
    w_qf = nc.dram_tensor("w_qf", [C.HID, C.QFN + C.QFR], BF16, kind="ExternalInput").ap()
    w_kva = nc.dram_tensor("w_kva", [C.HID, C.KVLR], BF16, kind="ExternalInput").ap()
    w_kbn = nc.dram_tensor("w_kbn", [C.KVLR, C.NH_G * C.DN], BF16, kind="ExternalInput").ap()
    w_kbr = nc.dram_tensor("w_kbr", [C.KVLR, C.NH_G * C.DR], BF16, kind="ExternalInput").ap()
    w_vb = nc.dram_tensor("w_vb", [C.KVLR, C.NH_G * C.DV], BF16, kind="ExternalInput").ap()
    w_ob = nc.dram_tensor("w_ob", [C.NH_G * C.DV, C.HID], BF16, kind="ExternalInput").ap()
    cos2 = nc.dram_tensor("cos2", [P, C.S], BF16, kind="ExternalInput").ap()
    ssin2 = nc.dram_tensor("ssin2", [P, C.S], BF16, kind="ExternalInput").ap()
    dmask = nc.dram_tensor("dmask", [P, P], BF16, kind="ExternalInput").ap()
    swp = nc.dram_tensor("swp", [P, P], BF16, kind="ExternalInput").ap()
    outp = nc.dram_tensor("outp", [C.S, C.HID], BF16, kind="ExternalOutput").ap()

    with tile.TileContext(nc) as tc, ExitStack() as wctx:
        # ---- weights + tables: loaded once, shared by all reps ----
        w_pool = wctx.enter_context(tc.tile_pool(name="wts", bufs=1))
        cos_sb = w_pool.tile([P, C.S], BF16)
        ssin_sb = w_pool.tile([P, C.S], BF16)
        dm_sb = w_pool.tile([P, P], BF16)
        swp_sb = w_pool.tile([P, P], BF16)
        wqf_sb = w_pool.tile([P, HO, C.QFN + C.QFR], BF16)
        wkva_sb = w_pool.tile([P, HO, C.KVLR], BF16)
        wkn_sb = w_pool.tile([P, KVC, C.NH_G * C.DN], BF16)
        wkr_sb = w_pool.tile([P, KVC, C.NH_G * C.DR], BF16)
        wv_sb = w_pool.tile([P, KVC, C.NH_G * C.DV], BF16)
        wo_sb = w_pool.tile([P, C.NH_G, C.HID], BF16)
        wqf_r = w_qf.rearrange("(ho hi) c -> hi ho c", hi=P)
        wkva_r = w_kva.rearrange("(ho hi) c -> hi ho c", hi=P)
        for ho in range(HO):
            nc.sync.dma_start(out=wqf_sb[:, ho, :], in_=wqf_r[:, ho, :])
        for ho in range(HO):
            nc.sync.dma_start(out=wkva_sb[:, ho, :], in_=wkva_r[:, ho, :])
        nc.sync.dma_start(out=cos_sb[:], in_=cos2)
        nc.sync.dma_start(out=ssin_sb[:], in_=ssin2)
        nc.sync.dma_start(out=swp_sb[:], in_=swp)
        nc.sync.dma_start(
            out=wkn_sb[:], in_=w_kbn.rearrange("(co ci) m -> ci co m", ci=P))
        nc.sync.dma_start(
            out=wkr_sb[:], in_=w_kbr.rearrange("(co ci) m -> ci co m", ci=P))
        nc.sync.dma_start(
            out=wv_sb[:], in_=w_vb.rearrange("(co ci) m -> ci co m", ci=P))
        nc.sync.dma_start(out=dm_sb[:], in_=dmask)
        nc.sync.dma_start(
            out=wo_sb[:], in_=w_ob.rearrange("(h d) o -> d h o", d=P))

        ht_pool = wctx.enter_context(tc.tile_pool(name="ht", bufs=2))
        hT_r = hT.rearrange("(ho hi) s -> hi ho s", hi=P)

        def load_ht(s0):
            # split into 4 sub-DMAs: finer dependency granularity (compute
            # starts after the first quarter) and the DMA-engine FIFO can
            # interleave weight-chunk loads.
            ht_sb = ht_pool.tile([P, HO, C.ST], BF16, tag="ht")
            for hq in range(0, HO, 4):
                nc.scalar.dma_start(
                    out=ht_sb[:, hq:hq + 4, :],
                    in_=hT_r[:, hq:hq + 4, s0:s0 + C.ST])
            return ht_sb

        # block-0 prefetch handle carried across reps: issued near the end of
        # rep r's P1 so the load lands during rep r's attention phase and
        # rep r+1's first matmul never waits on it.
        ht_next = [None]

        for rep in range(reps):
            with ExitStack() as tctx:
                # ---- persistent tiles (live across all phases) ----
                per_pool = tctx.enter_context(tc.tile_pool(name=f"persist{rep}", bufs=1))
                qTn_sb = per_pool.tile([P, NQN, C.S], BF16)
                qTr_sb = per_pool.tile([P, NPAIR, C.S], BF16)
                kTn_sb = per_pool.tile([P, C.NH_G, C.S], BF16)
                kTr_sb = per_pool.tile([P, NPAIR, C.S], BF16)
                v_sb = per_pool.tile([P, NVS, C.NH_G * C.DV], BF16)
                def rope_psum(rp_pool, rps_pool, ps_nat, dst_ap, s0, slen):
                    """psum of rope rows -> roped into dst (PE perm matmul)."""
                    tmp = rp_pool.tile([P, slen], BF16, tag="rope_tmp")
                    nc.scalar.copy(tmp[:], ps_nat[:])
                    ps2 = rps_pool.tile([P, slen], F32, tag="rope_ps2")
                    nc.tensor.matmul(ps2[:], swp_sb[:], tmp[:], start=True, stop=True)
                    m1 = rp_pool.tile([P, slen], BF16, tag="rope_m1")
                    nc.vector.tensor_mul(m1[:], tmp[:], cos_sb[:, s0:s0 + slen])
                    t2 = rp_pool.tile([P, slen], BF16, tag="rope_t2")
                    nc.vector.tensor_mul(t2[:], ps2[:], ssin_sb[:, s0:s0 + slen])
                    nc.vector.tensor_add(dst_ap, m1[:], t2[:])

                # ===== Phases 1+2 interleaved per s-block =====
                with ExitStack() as ctx:
                    rp_pool = ctx.enter_context(tc.tile_pool(name=f"rp{rep}", bufs=3))
                    kv_pool = ctx.enter_context(tc.tile_pool(name=f"kv{rep}", bufs=2))
                    ps1_pool = ctx.enter_context(
                        tc.tile_pool(name=f"ps1{rep}", bufs=3, space="PSUM"))
                    ps2_pool = ctx.enter_context(
                        tc.tile_pool(name=f"ps2{rep}", bufs=3, space="PSUM"))
                    rps_pool = ctx.enter_context(
                        tc.tile_pool(name=f"rps{rep}", bufs=2, space="PSUM"))

                    ht0_sb = ht_next[0] if ht_next[0] is not None else load_ht(0)

                    for st in range(NS):
                        s0 = st * C.ST
                        ht_sb = ht0_sb if st == 0 else load_ht(s0)

                        def accum(lhs_sb, col0):
                            ps = ps1_pool.tile([P, C.ST], F32, tag="ps1")
                            for h in range(HO):
                                nc.tensor.matmul(
                                    ps[:], lhs_sb[:, h, col0:col0 + P],
                                    ht_sb[:, h, :],
                                    start=(h == 0), stop=(h == HO - 1))
                            return ps

                        # --- phase 1: q projections + kv latent ---
                        for t in range(NQN):
                            ps = accum(wqf_sb, t * P)
                            nc.scalar.copy(qTn_sb[:, t, s0:s0 + C.ST], ps[:])
                        for pr in range(NPAIR):
                            ps_nat = accum(wqf_sb, C.QFN + pr * P)
                            rope_psum(rp_pool, rps_pool, ps_nat,
                                      qTr_sb[:, pr, s0:s0 + C.ST], s0, C.ST)
                        kv_sb = kv_pool.tile([P, KVC, C.ST], BF16, tag="kv")
                        for cc in range(KVC):
                            ps = accum(wkva_sb, cc * P)
                            nc.vector.tensor_copy(kv_sb[:, cc, :], ps[:])

                        # --- phase 2: k/v head projections from kv latent ---
                        for h in range(C.NH_G):
                            ps = ps2_pool.tile([P, C.ST], F32, tag="p2")
                            for cch in range(KVC):
                                nc.tensor.matmul(
                                    ps[:], wkn_sb[:, cch, h * C.DN:(h + 1) * C.DN],
                                    kv_sb[:, cch, :],
                                    start=(cch == 0), stop=(cch == KVC - 1))
                            nc.scalar.copy(kTn_sb[:, h, s0:s0 + C.ST], ps[:])
                        for pr in range(NPAIR):
                            ps_nat = ps2_pool.tile([P, C.ST], F32, tag="p2")
                            for cch in range(KVC):
                                nc.tensor.matmul(
                                    ps_nat[:], wkr_sb[:, cch, pr * P:(pr + 1) * P],
                                    kv_sb[:, cch, :],
                                    start=(cch == 0), stop=(cch == KVC - 1))
                            rope_psum(rp_pool, rps_pool, ps_nat,
                                      kTr_sb[:, pr, s0:s0 + C.ST], s0, C.ST)
                        for ssub in range(C.ST // P):
                            vs = (s0 + ssub * P) // P
                            ps = ps2_pool.tile([P, C.NH_G * C.DV], F32, tag="p2")
                            for cch in range(KVC):
                                nc.tensor.matmul(
                                    ps[:], kv_sb[:, cch, ssub * P:(ssub + 1) * P],
                                    wv_sb[:, cch, :],
                                    start=(cch == 0), stop=(cch == KVC - 1))
                            nc.scalar.copy(v_sb[:, vs, :], ps[:])

                ht_next[0] = load_ht(0) if rep < reps - 1 else None

                # ================= Phase 3: attention + w_o =================
                with ExitStack() as ctx:
                    e_pool = ctx.enter_context(tc.tile_pool(name=f"ae{rep}", bufs=4))
                    es_pool = ctx.enter_context(tc.tile_pool(name=f"aes{rep}", bufs=1))
                    d_pool = ctx.enter_context(tc.tile_pool(name=f"ad{rep}", bufs=1))
                    ao_pool = ctx.enter_context(tc.tile_pool(name=f"aao{rep}", bufs=2))
                    oev_pool = ctx.enter_context(tc.tile_pool(name=f"aoe{rep}", bufs=2))
                    ps_s = ctx.enter_context(
                        tc.tile_pool(name=f"apss{rep}", bufs=4, space="PSUM"))
                    ps_o = ctx.enter_context(
                        tc.tile_pool(name=f"apso{rep}", bufs=2, space="PSUM"))

                    def emit_wo(qt, ao_sb):
                        q0 = qt * C.QT
                        for qs in range(C.QT // P):
                            oev = oev_pool.tile([P, C.HID], BF16, tag="oev")
                            for ot in range(NOT):
                                psw = ps_s.tile([P, C.QT], F32, tag="pss")
                                for h in range(C.NH_G):
                                    nc.tensor.matmul(
                                        psw[:], ao_sb[:, h, qs * P:(qs + 1) * P],
                                        wo_sb[:, h, ot * 512:(ot + 1) * 512],
                                        start=(h == 0), stop=(h == C.NH_G - 1))
                                nc.scalar.copy(
                                    oev[:, ot * 512:(ot + 1) * 512], psw[:])
                            nc.sync.dma_start(
                                out=outp[q0 + qs * P:q0 + (qs + 1) * P, :],
                                in_=oev[:])

                    wo_prev = None
                    for qt in range(NQT):
                        q0 = qt * C.QT
                        nkt = (qt + 1) * C.QT // P
                        ao_sb = ao_pool.tile([P, C.NH_G, C.QT], BF16, tag="ao")
                        for pr in range(NPAIR):
                            # the two heads of a pair run interleaved through
                            # the k-loop: doubles pipeline depth so act/vector
                            # latency hides behind the PE even at small nkt.
                            esum = [es_pool.tile([P, C.QT], F32R, tag=f"es{hh}",
                                                 name=f"esum{hh}")
                                    for hh in range(2)]
                            pso = [ps_o.tile([P, C.QT], F32, tag=f"pso{hh}",
                                             name=f"pso{hh}")
                                   for hh in range(2)]

                            def consume_part(hh, e_prev, ktp, qoff):
                                nc.tensor.matmul(
                                    pso[hh][:, qoff:C.QT],
                                    v_sb[:, ktp, (pr * 2 + hh) * C.DV:
                                         (pr * 2 + hh + 1) * C.DV],
                                    e_prev[:, qoff:C.QT],
                                    start=(ktp == 0), stop=(ktp == nkt - 1))

                            pend = [[], []]
                            for kt in range(nkt):
                                k0 = kt * P
                                # diagonal k-tiles only need q >= k: shrink
                                # the moving dim to the surviving q-range.
                                j = kt - qt * NDIAG
                                qoff = max(j, 0) * P
                                for hh in range(2):
                                    h = pr * 2 + hh
                                    rsl = slice(hh * C.DR, (hh + 1) * C.DR)
                                    pss = ps_s.tile([P, C.QT], F32, tag="pss")
                                    nc.tensor.matmul(
                                        pss[:, qoff:C.QT], kTn_sb[:, h, k0:k0 + P],
                                        qTn_sb[:, h, q0 + qoff:q0 + C.QT],
                                        start=True, stop=False)
                                    nc.tensor.matmul(
                                        pss[:, qoff:C.QT], kTr_sb[rsl, pr, k0:k0 + P],
                                        qTr_sb[rsl, pr, q0 + qoff:q0 + C.QT],
                                        start=False, stop=True)
                                    e_sb = e_pool.tile([P, C.QT], BF16, tag=f"e{hh}")
                                    nc.scalar.activation(
                                        e_sb[:, qoff:C.QT], pss[:, qoff:C.QT],
                                        mybir.ActivationFunctionType.Exp,
                                        scale=C.SCALE)
                                    if j >= 0:
                                        nc.gpsimd.tensor_mul(
                                            e_sb[:, qoff:qoff + P],
                                            e_sb[:, qoff:qoff + P], dm_sb[:])
                                    if kt == 0:
                                        nc.vector.tensor_copy(esum[hh][:], e_sb[:])
                                    else:
                                        nc.vector.tensor_add(
                                            esum[hh][:, qoff:C.QT],
                                            esum[hh][:, qoff:C.QT],
                                            e_sb[:, qoff:C.QT])
                                    # consume two iterations behind: the
                                    # V-matmul's wait on e is then satisfied
                                    # well before the PE reaches it.
                                    if len(pend[hh]) >= 2:
                                        consume_part(hh, *pend[hh].pop(0))
                                    pend[hh].append((e_sb, kt, qoff))
                            for hh in range(2):
                                h = pr * 2 + hh
                                while pend[hh]:
                                    consume_part(hh, *pend[hh].pop(0))
                                dsum = d_pool.tile([P, C.QT], F32, tag=f"ds{hh}")
                                nc.gpsimd.partition_all_reduce(
                                    dsum[:], esum[hh][:], channels=P,
                                    reduce_op=bass_isa.ReduceOp.add)
                                rec = d_pool.tile([P, C.QT], F32, tag=f"rec{hh}")
                                nc.vector.reciprocal(rec[:], dsum[:])
                                nc.vector.tensor_mul(
                                    ao_sb[:, h, :], pso[hh][:], rec[:])
                        if wo_prev is not None:
                            emit_wo(*wo_prev)
                        wo_prev = (qt, ao_sb)
                    emit_wo(*wo_prev)

    nc.compile()
    return nc


def rope_tables(C: Cfg):
    """cos2/ssin2 [128, S]: two stacked 64-row blocks (head pairs share).

    ssin carries the rotate_half sign: rows [0:32] (and [64:96]) negated.
    """
    inv = 1.0 / (10000.0 ** (np.arange(0, C.DR, 2, dtype=np.float64) / C.DR))
    freqs = np.arange(C.S, dtype=np.float64)[:, None] * inv[None, :]  # [S, 32]
    emb = np.concatenate([freqs, freqs], axis=1)  # [S, 64]
    cos = np.cos(emb).T.astype(np.float32)   # [64, S]
    sin = np.sin(emb).T.astype(np.float32)
    ssin = sin.copy()
    ssin[: C.DR // 2] = -ssin[: C.DR // 2]
    cos2 = np.concatenate([cos, cos], axis=0)     # [128, S]
    ssin2 = np.concatenate([ssin, ssin], axis=0)
    return np.ascontiguousarray(cos2), np.ascontiguousarray(ssin2)


def host_inputs(C: Cfg, inputs: dict, core: int):
    """Build the per-core input map from full inputs (cast to bf16)."""
    import ml_dtypes

    bf16 = ml_dtypes.bfloat16
    NH = inputs["w_q_nope"].shape[1] // C.DN
    groups = NH // C.NH_G
    b = core // groups
    g = core % groups
    hs = slice(g * C.NH_G, (g + 1) * C.NH_G)

    cast = lambda x: np.ascontiguousarray(np.asarray(x, dtype=np.float32).astype(bf16))

    hT = cast(inputs["hidden_states"][b].T)
    w_q_a = np.asarray(inputs["w_q_a"], dtype=np.float32)
    w_qbn = np.asarray(inputs["w_q_nope"], dtype=np.float32).reshape(
        C.QLR, NH, C.DN)[:, hs].reshape(C.QLR, -1)
    w_qbr = np.asarray(inputs["w_q_rope"], dtype=np.float32).reshape(
        C.QLR, NH, C.DR)[:, hs].reshape(C.QLR, -1)
    w_qf = cast(np.concatenate([w_q_a @ w_qbn, w_q_a @ w_qbr], axis=1))
    w_kva = cast(inputs["w_kv_a"])
    w_kbn = cast(np.asarray(inputs["w_k_nope"], dtype=np.float32).reshape(
        C.KVLR, NH, C.DN)[:, hs].reshape(C.KVLR, -1))
    w_kbr = cast(np.asarray(inputs["w_k_rope"], dtype=np.float32).reshape(
        C.KVLR, NH, C.DR)[:, hs].reshape(C.KVLR, -1))
    w_vb = cast(np.asarray(inputs["w_v"], dtype=np.float32).reshape(
        C.KVLR, NH, C.DV)[:, hs].reshape(C.KVLR, -1))
    w_ob = cast(np.asarray(inputs["w_o"], dtype=np.float32).reshape(
        NH, C.DV, C.HID)[hs].reshape(-1, C.HID))
    cos2, ssin2 = rope_tables(C)
    # [k, q] triangle for one 128x128 diagonal sub-block: keep q >= k
    dmask = np.triu(np.ones((128, 128), dtype=np.float32))
    swp = np.zeros((128, 128), dtype=np.float32)
    for k in range(128):
        swp[k, k ^ 32] = 1.0
    return {
        "hT": hT, "w_qf": w_qf, "w_kva": w_kva,
        "w_kbn": w_kbn, "w_kbr": w_kbr, "w_vb": w_vb, "w_ob": w_ob,
        "cos2": cast(cos2), "ssin2": cast(ssin2),
        "dmask": cast(dmask), "swp": cast(swp),
    }


_NC_CACHE = {}


def kernel(**inputs) -> np.ndarray:
    from concourse.bass_utils import run_bass_kernel_spmd

    C = CFG
    if "nc" not in _NC_CACHE:
        _NC_CACHE["nc"] = build_nc(C)
    nc = _NC_CACHE["nc"]

    in_maps = [host_inputs(C, inputs, c) for c in range(8)]
    res = run_bass_kernel_spmd(nc, in_maps, core_ids=list(range(8)))

    B = inputs["hidden_states"].shape[0]
    groups = 8 // B
    out = np.zeros((B, C.S, C.HID), dtype=np.float32)
    for c in range(8):
        out[c // groups] += np.asarray(res.results[c]["outp"], dtype=np.float32)
    return out


# revision 29
# speedup vs baseline: 1.3246x; 1.3246x over previous
"""DeepseekV3 MLA attention kernel for 8 Trainium2 NeuronCores.

Sharding: 2-way data-parallel over batch x 4-way tensor-parallel over heads.
Core c handles batch b = c // 4 and heads [4*(c%4) .. 4*(c%4)+4).

All matmul operands are bf16 (psum accumulation stays f32); intermediates
(qT, kv latent, kT, v) are SBUF-resident for the whole kernel -- no DRAM
scratch round-trips. Weights and tables load once and are shared across
timing reps. Per s-block the q/kv projections (phase 1) and the k/v head
projections (phase 2) are interleaved so the PE never drains. Attention
(phase 3) runs the two heads of a rope-pair interleaved through the
k-loop (pipeline depth), with the scoresT layout and max-free softmax:
exp on the scalar engine, causal mask + denominator accumulation on the
vector engine, the partition reduction via gpsimd partition_all_reduce,
and 1/denominator scaling on the vector engine. Diagonal k-tiles shrink
their moving dim to the surviving causal q-range. RoPE rotate_half is a
128x128 permutation matmul on the PE. The w_o stage for q-tile t is
emitted after the attention of q-tile t+1 so its softmax tail chains
hide under PE work. Host sums the 4 partial outputs per batch.
"""

from contextlib import ExitStack
from dataclasses import dataclass

import numpy as np

import concourse.bacc as bacc
import concourse.mybir as mybir
import concourse.tile as tile
from concourse import bass_isa

F32 = mybir.dt.float32
F32R = mybir.dt.float32r
BF16 = mybir.dt.bfloat16


@dataclass(frozen=True)
class Cfg:
    S: int = 2048          # sequence length (per batch)
    HID: int = 2048        # hidden dim
    QLR: int = 1536        # q lora rank (host-side only)
    KVLR: int = 512        # kv lora rank
    NH_G: int = 4          # heads per core
    DN: int = 128          # nope dim
    DR: int = 64           # rope dim
    DV: int = 128          # v head dim
    ST: int = 512          # phase-1/2 s-block width
    QT: int = 512          # attention q-tile width

    @property
    def QFN(self):
        return self.NH_G * self.DN      # fused q nope cols

    @property
    def QFR(self):
        return (self.NH_G // 2) * 128   # fused q rope cols (pair-packed)

    @property
    def SCALE(self):
        return 1.0 / float(np.sqrt(self.DN + self.DR))


CFG = Cfg()


def build_nc(C: Cfg, reps: int = 1):
    nc = bacc.Bacc("TRN2", target_bir_lowering=False, debug=False, num_devices=8)
    P = 128
    HO = C.HID // P
    NS = C.S // C.ST
    KVC = C.KVLR // P
    NPAIR = C.NH_G // 2
    NQT = C.S // C.QT
    NDIAG = C.QT // P
    NVS = C.S // P
    NQN = C.QFN // P
    NOT = C.HID // 512

    # ---- kernel I/O (bf16 unless noted) ----
    hT = nc.dram_tensor("hT", [C.HID, C.S], BF16, kind="ExternalInput").ap()
    w_qf = nc.dram_tensor("w_qf", [C.HID, C.QFN + C.QFR], BF16, kind="ExternalInput").ap()
    w_kva = nc.dram_tensor("w_kva", [C.HID, C.KVLR], BF16, kind="ExternalInput").ap()
    w_kbn = nc.dram_tensor("w_kbn", [C.KVLR, C.NH_G * C.DN], BF16, kind="ExternalInput").ap()
    w_kbr = nc.dram_tensor("w_kbr", [C.KVLR, C.NH_G * C.DR], BF16, kind="ExternalInput").ap()
    w_vb = nc.dram_tensor("w_vb", [C.KVLR, C.NH_G * C.DV], BF16, kind="ExternalInput").ap()
    w_ob = nc.dram_tensor("w_ob", [C.NH_G * C.DV, C.HID], BF16, kind="ExternalInput").ap()
    cos2 = nc.dram_tensor("cos2", [P, C.S], BF16, kind="ExternalInput").ap()
    ssin2 = nc.dram_tensor("ssin2", [P, C.S], BF16, kind="ExternalInput").ap()
    dmask = nc.dram_tensor("dmask", [P, P], BF16, kind="ExternalInput").ap()
    swp = nc.dram_tensor("swp", [P, P], BF16, kind="ExternalInput").ap()
    outp = nc.dram_tensor("outp", [C.S, C.HID], BF16, kind="ExternalOutput").ap()

    with tile.TileContext(nc) as tc, ExitStack() as wctx:
        # ---- weights + tables: loaded once, shared by all reps ----
        w_pool = wctx.enter_context(tc.tile_pool(name="wts", bufs=1))
        cos_sb = w_pool.tile([P, C.S], BF16)
        ssin_sb = w_pool.tile([P, C.S], BF16)
        dm_sb = w_pool.tile([P, P], BF16)
        swp_sb = w_pool.tile([P, P], BF16)
        wqf_sb = w_pool.tile([P, HO, C.QFN + C.QFR], BF16)
        wkva_sb = w_pool.tile([P, HO, C.KVLR], BF16)
        wkn_sb = w_pool.tile([P, KVC, C.NH_G * C.DN], BF16)
        wkr_sb = w_pool.tile([P, KVC, C.NH_G * C.DR], BF16)
        wv_sb = w_pool.tile([P, KVC, C.NH_G * C.DV], BF16)
        wo_sb = w_pool.tile([P, C.NH_G, C.HID], BF16)
        wqf_r = w_qf.rearrange("(ho hi) c -> hi ho c", hi=P)
        wkva_r = w_kva.rearrange("(ho hi) c -> hi ho c", hi=P)
        for ho in range(HO):
            nc.sync.dma_start(out=wqf_sb[:, ho, :], in_=wqf_r[:, ho, :])
        for ho in range(HO):
            nc.sync.dma_start(out=wkva_sb[:, ho, :], in_=wkva_r[:, ho, :])
        nc.sync.dma_start(out=cos_sb[:], in_=cos2)
        nc.sync.dma_start(out=ssin_sb[:], in_=ssin2)
        nc.sync.dma_start(out=swp_sb[:], in_=swp)
        nc.sync.dma_start(
            out=wkn_sb[:], in_=w_kbn.rearrange("(co ci) m -> ci co m", ci=P))
        nc.sync.dma_start(
            out=wkr_sb[:], in_=w_kbr.rearrange("(co ci) m -> ci co m", ci=P))
        nc.sync.dma_start(
            out=wv_sb[:], in_=w_vb.rearrange("(co ci) m -> ci co m", ci=P))
        nc.sync.dma_start(out=dm_sb[:], in_=dmask)
        nc.sync.dma_start(
            out=wo_sb[:], in_=w_ob.rearrange("(h d) o -> d h o", d=P))

        ht_pool = wctx.enter_context(tc.tile_pool(name="ht", bufs=2))
        hT_r = hT.rearrange("(ho hi) s -> hi ho s", hi=P)

        def load_ht(s0):
            # split into 4 sub-DMAs: finer dependency granularity (compute
            # starts after the first quarter) and the DMA-engine FIFO can
            # interleave weight-chunk loads.
            ht_sb = ht_pool.tile([P, HO, C.ST], BF16, tag="ht")
            for hq in range(0, HO, 4):
                nc.scalar.dma_start(
                    out=ht_sb[:, hq:hq + 4, :],
                    in_=hT_r[:, hq:hq + 4, s0:s0 + C.ST])
            return ht_sb

        # block-0 prefetch handle carried across reps: issued near the end of
        # rep r's P1 so the load lands during rep r's attention phase and
        # rep r+1's first matmul never waits on it.
        ht_next = [None]

        for rep in range(reps):
            with ExitStack() as tctx:
                # ---- persistent tiles (live across all phases) ----
                per_pool = tctx.enter_context(tc.tile_pool(name=f"persist{rep}", bufs=1))
                qTn_sb = per_pool.tile([P, NQN, C.S], BF16)
                qTr_sb = per_pool.tile([P, NPAIR, C.S], BF16)
                kTn_sb = per_pool.tile([P, C.NH_G, C.S], BF16)
                kTr_sb = per_pool.tile([P, NPAIR, C.S], BF16)
                v_sb = per_pool.tile([P, NVS, C.NH_G * C.DV], BF16)
                def rope_psum(rp_pool, rps_pool, ps_nat, dst_ap, s0, slen):
                    """psum of rope rows -> roped into dst (PE perm matmul)."""
                    tmp = rp_pool.tile([P, slen], BF16, tag="rope_tmp")
                    nc.scalar.copy(tmp[:], ps_nat[:])
                    ps2 = rps_pool.tile([P, slen], F32, tag="rope_ps2")
                    nc.tensor.matmul(ps2[:], swp_sb[:], tmp[:], start=True, stop=True)
                    m1 = rp_pool.tile([P, slen], BF16, tag="rope_m1")
                    nc.vector.tensor_mul(m1[:], tmp[:], cos_sb[:, s0:s0 + slen])
                    t2 = rp_pool.tile([P, slen], BF16, tag="rope_t2")
                    nc.vector.tensor_mul(t2[:], ps2[:], ssin_sb[:, s0:s0 + slen])
                    nc.vector.tensor_add(dst_ap, m1[:], t2[:])

                # ===== Phases 1+2 interleaved per s-block =====
                with ExitStack() as ctx:
                    rp_pool = ctx.enter_context(tc.tile_pool(name=f"rp{rep}", bufs=3))
                    kv_pool = ctx.enter_context(tc.tile_pool(name=f"kv{rep}", bufs=2))
                    ps1_pool = ctx.enter_context(
                        tc.tile_pool(name=f"ps1{rep}", bufs=3, space="PSUM"))
                    ps2_pool = ctx.enter_context(
                        tc.tile_pool(name=f"ps2{rep}", bufs=3, space="PSUM"))
                    rps_pool = ctx.enter_context(
                        tc.tile_pool(name=f"rps{rep}", bufs=2, space="PSUM"))

                    ht0_sb = ht_next[0] if ht_next[0] is not None else load_ht(0)

                    for st in range(NS):
                        s0 = st * C.ST
                        ht_sb = ht0_sb if st == 0 else load_ht(s0)

                        def accum(lhs_sb, col0):
                            ps = ps1_pool.tile([P, C.ST], F32, tag="ps1")
                            for h in range(HO):
                                nc.tensor.matmul(
                                    ps[:], lhs_sb[:, h, col0:col0 + P],
                                    ht_sb[:, h, :],
                                    start=(h == 0), stop=(h == HO - 1))
                            return ps

                        # --- phase 1: q projections + kv latent ---
                        for t in range(NQN):
                            ps = accum(wqf_sb, t * P)
                            nc.scalar.copy(qTn_sb[:, t, s0:s0 + C.ST], ps[:])
                        for pr in range(NPAIR):
                            ps_nat = accum(wqf_sb, C.QFN + pr * P)
                            rope_psum(rp_pool, rps_pool, ps_nat,
                                      qTr_sb[:, pr, s0:s0 + C.ST], s0, C.ST)
                        kv_sb = kv_pool.tile([P, KVC, C.ST], BF16, tag="kv")
                        for cc in range(KVC):
                            ps = accum(wkva_sb, cc * P)
                            nc.vector.tensor_copy(kv_sb[:, cc, :], ps[:])

                        # --- phase 2: k/v head projections from kv latent ---
                        for h in range(C.NH_G):
                            ps = ps2_pool.tile([P, C.ST], F32, tag="p2")
                            for cch in range(KVC):
                                nc.tensor.matmul(
                                    ps[:], wkn_sb[:, cch, h * C.DN:(h + 1) * C.DN],
                                    kv_sb[:, cch, :],
                                    start=(cch == 0), stop=(cch == KVC - 1))
                            nc.scalar.copy(kTn_sb[:, h, s0:s0 + C.ST], ps[:])
                        for pr in range(NPAIR):
                            ps_nat = ps2_pool.tile([P, C.ST], F32, tag="p2")
                            for cch in range(KVC):
                                nc.tensor.matmul(
                                    ps_nat[:], wkr_sb[:, cch, pr * P:(pr + 1) * P],
                                    kv_sb[:, cch, :],
                                    start=(cch == 0), stop=(cch == KVC - 1))
                            rope_psum(rp_pool, rps_pool, ps_nat,
                                      kTr_sb[:, pr, s0:s0 + C.ST], s0, C.ST)
                        for ssub in range(C.ST // P):
                            vs = (s0 + ssub * P) // P
                            ps = ps2_pool.tile([P, C.NH_G * C.DV], F32, tag="p2")
                            for cch in range(KVC):
                                nc.tensor.matmul(
                                    ps[:], kv_sb[:, cch, ssub * P:(ssub + 1) * P],
                                    wv_sb[:, cch, :],
                                    start=(cch == 0), stop=(cch == KVC - 1))
                            nc.scalar.copy(v_sb[:, vs, :], ps[:])

                ht_next[0] = load_ht(0) if rep < reps - 1 else None

                # ================= Phase 3: attention + w_o =================
                with ExitStack() as ctx:
                    e_pool = ctx.enter_context(tc.tile_pool(name=f"ae{rep}", bufs=4))
                    es_pool = ctx.enter_context(tc.tile_pool(name=f"aes{rep}", bufs=1))
                    d_pool = ctx.enter_context(tc.tile_pool(name=f"ad{rep}", bufs=1))
                    ao_pool = ctx.enter_context(tc.tile_pool(name=f"aao{rep}", bufs=2))
                    oev_pool = ctx.enter_context(tc.tile_pool(name=f"aoe{rep}", bufs=2))
                    ps_s = ctx.enter_context(
                        tc.tile_pool(name=f"apss{rep}", bufs=4, space="PSUM"))
                    ps_o = ctx.enter_context(
                        tc.tile_pool(name=f"apso{rep}", bufs=2, space="PSUM"))

                    def emit_wo(qt, ao_sb):
                        q0 = qt * C.QT
                        for qs in range(C.QT // P):
                            oev = oev_pool.tile([P, C.HID], BF16, tag="oev")
                            for ot in range(NOT):
                                psw = ps_s.tile([P, C.QT], F32, tag="pss")
                                for h in range(C.NH_G):
                                    nc.tensor.matmul(
                                        psw[:], ao_sb[:, h, qs * P:(qs + 1) * P],
                                        wo_sb[:, h, ot * 512:(ot + 1) * 512],
                                        start=(h == 0), stop=(h == C.NH_G - 1))
                                nc.scalar.copy(
                                    oev[:, ot * 512:(ot + 1) * 512], psw[:])
                            nc.sync.dma_start(
                                out=outp[q0 + qs * P:q0 + (qs + 1) * P, :],
                                in_=oev[:])

                    wo_prev = None
                    for qt in range(NQT):
                        q0 = qt * C.QT
                        nkt = (qt + 1) * C.QT // P
                        ao_sb = ao_pool.tile([P, C.NH_G, C.QT], BF16, tag="ao")
                        for pr in range(NPAIR):
                            # the two heads of a pair run interleaved through
                            # the k-loop: doubles pipeline depth so act/vector
                            # latency hides behind the PE even at small nkt.
                            esum = [es_pool.tile([P, C.QT], F32R, tag=f"es{hh}",
                                                 name=f"esum{hh}")
                                    for hh in range(2)]
                            pso = [ps_o.tile([P, C.QT], F32, tag=f"pso{hh}",
                                             name=f"pso{hh}")
                                   for hh in range(2)]

                            def consume_part(hh, e_prev, ktp, qoff):
                                nc.tensor.matmul(
                                    pso[hh][:, qoff:C.QT],
                                    v_sb[:, ktp, (pr * 2 + hh) * C.DV:
                                         (pr * 2 + hh + 1) * C.DV],
                                    e_prev[:, qoff:C.QT],
                                    start=(ktp == 0), stop=(ktp == nkt - 1))

                            pend = [[], []]
                            for kt in range(nkt):
                                k0 = kt * P
                                # diagonal k-tiles only need q >= k: shrink
                                # the moving dim to the surviving q-range.
                                j = kt - qt * NDIAG
                                qoff = max(j, 0) * P
                                for hh in range(2):
                                    h = pr * 2 + hh
                                    rsl = slice(hh * C.DR, (hh + 1) * C.DR)
                                    pss = ps_s.tile([P, C.QT], F32, tag="pss")
                                    nc.tensor.matmul(
                                        pss[:, qoff:C.QT], kTn_sb[:, h, k0:k0 + P],
                                        qTn_sb[:, h, q0 + qoff:q0 + C.QT],
                                        start=True, stop=False)
                                    nc.tensor.matmul(
                                        pss[:, qoff:C.QT], kTr_sb[rsl, pr, k0:k0 + P],
                                        qTr_sb[rsl, pr, q0 + qoff:q0 + C.QT],
                                        start=False, stop=True)
                                    e_sb = e_pool.tile([P, C.QT], BF16, tag=f"e{hh}")
                                    nc.scalar.activation(
                                        e_sb[:, qoff:C.QT], pss[:, qoff:C.QT],
                                        mybir.ActivationFunctionType.Exp,
                                        scale=C.SCALE)
                                    if j >= 0:
                                        nc.vector.tensor_mul(
                                            e_sb[:, qoff:qoff + P],
                                            e_sb[:, qoff:qoff + P], dm_sb[:])
                                    if kt == 0:
                                        nc.vector.tensor_copy(esum[hh][:], e_sb[:])
                                    else:
                                        nc.vector.tensor_add(
                                            esum[hh][:, qoff:C.QT],
                                            esum[hh][:, qoff:C.QT],
                                            e_sb[:, qoff:C.QT])
                                    # consume two iterations behind: the
                                    # V-matmul's wait on e is then satisfied
                                    # well before the PE reaches it.
                                    if len(pend[hh]) >= 2:
                                        consume_part(hh, *pend[hh].pop(0))
                                    pend[hh].append((e_sb, kt, qoff))
                            for hh in range(2):
                                h = pr * 2 + hh
                                while pend[hh]:
                                    consume_part(hh, *pend[hh].pop(0))
                                dsum = d_pool.tile([P, C.QT], F32, tag=f"ds{hh}")
                                nc.gpsimd.partition_all_reduce(
                                    dsum[:], esum[hh][:], channels=P,
                                    reduce_op=bass_isa.ReduceOp.add)
                                rec = d_pool.tile([P, C.QT], F32, tag=f"rec{hh}")
                                nc.vector.reciprocal(rec[:], dsum[:])
                                nc.vector.tensor_mul(
                                    ao_sb[:, h, :], pso[hh][:], rec[:])
                        if wo_prev is not None:
                            emit_wo(*wo_prev)
                        wo_prev = (qt, ao_sb)
                    emit_wo(*wo_prev)

    nc.compile()
    return nc


def rope_tables(C: Cfg):
    """cos2/ssin2 [128, S]: two stacked 64-row blocks (head pairs share).

    ssin carries the rotate_half sign: rows [0:32] (and [64:96]) negated.
    """
    inv = 1.0 / (10000.0 ** (np.arange(0, C.DR, 2, dtype=np.float64) / C.DR))
    freqs = np.arange(C.S, dtype=np.float64)[:, None] * inv[None, :]  # [S, 32]
    emb = np.concatenate([freqs, freqs], axis=1)  # [S, 64]
    cos = np.cos(emb).T.astype(np.float32)   # [64, S]
    sin = np.sin(emb).T.astype(np.float32)
    ssin = sin.copy()
    ssin[: C.DR // 2] = -ssin[: C.DR // 2]
    cos2 = np.concatenate([cos, cos], axis=0)     # [128, S]
    ssin2 = np.concatenate([ssin, ssin], axis=0)
    return np.ascontiguousarray(cos2), np.ascontiguousarray(ssin2)


def host_inputs(C: Cfg, inputs: dict, core: int):
    """Build the per-core input map from full inputs (cast to bf16)."""
    import ml_dtypes

    bf16 = ml_dtypes.bfloat16
    NH = inputs["w_q_nope"].shape[1] // C.DN
    groups = NH // C.NH_G
    b = core // groups
    g = core % groups
    hs = slice(g * C.NH_G, (g + 1) * C.NH_G)

    cast = lambda x: np.ascontiguousarray(np.asarray(x, dtype=np.float32).astype(bf16))

    hT = cast(inputs["hidden_states"][b].T)
    w_q_a = np.asarray(inputs["w_q_a"], dtype=np.float32)
    w_qbn = np.asarray(inputs["w_q_nope"], dtype=np.float32).reshape(
        C.QLR, NH, C.DN)[:, hs].reshape(C.QLR, -1)
    w_qbr = np.asarray(inputs["w_q_rope"], dtype=np.float32).reshape(
        C.QLR, NH, C.DR)[:, hs].reshape(C.QLR, -1)
    w_qf = cast(np.concatenate([w_q_a @ w_qbn, w_q_a @ w_qbr], axis=1))
    w_kva = cast(inputs["w_kv_a"])
    w_kbn = cast(np.asarray(inputs["w_k_nope"], dtype=np.float32).reshape(
        C.KVLR, NH, C.DN)[:, hs].reshape(C.KVLR, -1))
    w_kbr = cast(np.asarray(inputs["w_k_rope"], dtype=np.float32).reshape(
        C.KVLR, NH, C.DR)[:, hs].reshape(C.KVLR, -1))
    w_vb = cast(np.asarray(inputs["w_v"], dtype=np.float32).reshape(
        C.KVLR, NH, C.DV)[:, hs].reshape(C.KVLR, -1))
    w_ob = cast(np.asarray(inputs["w_o"], dtype=np.float32).reshape(
        NH, C.DV, C.HID)[hs].reshape(-1, C.HID))
    cos2, ssin2 = rope_tables(C)
    # [k, q] triangle for one 128x128 diagonal sub-block: keep q >= k
    dmask = np.triu(np.ones((128, 128), dtype=np.float32))
    swp = np.zeros((128, 128), dtype=np.float32)
    for k in range(128):
        swp[k, k ^ 32] = 1.0
    return {
        "hT": hT, "w_qf": w_qf, "w_kva": w_kva,
        "w_kbn": w_kbn, "w_kbr": w_kbr, "w_vb": w_vb, "w_ob": w_ob,
        "cos2": cast(cos2), "ssin2": cast(ssin2),
        "dmask": cast(dmask), "swp": cast(swp),
    }


_NC_CACHE = {}


def kernel(**inputs) -> np.ndarray:
    from concourse.bass_utils import run_bass_kernel_spmd

    C = CFG
    if "nc" not in _NC_CACHE:
        _NC_CACHE["nc"] = build_nc(C)
    nc = _NC_CACHE["nc"]

    in_maps = [host_inputs(C, inputs, c) for c in range(8)]
    res = run_bass_kernel_spmd(nc, in_maps, core_ids=list(range(8)))

    B = inputs["hidden_states"].shape[0]
    groups = 8 // B
    out = np.zeros((B, C.S, C.HID), dtype=np.float32)
    for c in range(8):
        out[c // groups] += np.asarray(res.results[c]["outp"], dtype=np.float32)
    return out


# revision 31
# speedup vs baseline: 1.6684x; 1.2595x over previous
"""DeepseekV3 MLA attention kernel for 8 Trainium2 NeuronCores.

Sharding: 2-way data-parallel over batch x 4-way tensor-parallel over heads.
Core c handles batch b = c // 4 and heads [4*(c%4) .. 4*(c%4)+4).

All matmul operands are bf16 (psum accumulation stays f32); intermediates
(qT, kv latent, kT, v) are SBUF-resident for the whole kernel -- no DRAM
scratch round-trips. Weights and tables load once and are shared across
timing reps. Per s-block the q/kv projections (phase 1) and the k/v head
projections (phase 2) are interleaved so the PE never drains. Attention
(phase 3) runs the two heads of a rope-pair interleaved through the
k-loop (pipeline depth), with the scoresT layout and max-free softmax:
exp on the scalar engine, causal mask + denominator accumulation on the
vector engine, the partition reduction via gpsimd partition_all_reduce,
and 1/denominator scaling on the vector engine. Diagonal k-tiles shrink
their moving dim to the surviving causal q-range. RoPE rotate_half is a
128x128 permutation matmul on the PE. The w_o stage for q-tile t is
emitted after the attention of q-tile t+1 so its softmax tail chains
hide under PE work. Host sums the 4 partial outputs per batch.
"""

from contextlib import ExitStack
from dataclasses import dataclass

import numpy as np

import concourse.bacc as bacc
import concourse.mybir as mybir
import concourse.tile as tile
from concourse import bass_isa

F32 = mybir.dt.float32
F32R = mybir.dt.float32r
BF16 = mybir.dt.bfloat16


@dataclass(frozen=True)
class Cfg:
    S: int = 2048          # sequence length (per batch)
    HID: int = 2048        # hidden dim
    QLR: int = 1536        # q lora rank (host-side only)
    KVLR: int = 512        # kv lora rank
    NH_G: int = 4          # heads per core
    DN: int = 128          # nope dim
    DR: int = 64           # rope dim
    DV: int = 128          # v head dim
    ST: int = 512          # phase-1/2 s-block width
    QT: int = 512          # attention q-tile width

    @property
    def QFN(self):
        return self.NH_G * self.DN      # fused q nope cols

    @property
    def QFR(self):
        return (self.NH_G // 2) * 128   # fused q rope cols (pair-packed)

    @property
    def SCALE(self):
        return 1.0 / float(np.sqrt(self.DN + self.DR))


CFG = Cfg()


def build_nc(C: Cfg, reps: int = 1):
    nc = bacc.Bacc("TRN2", target_bir_lowering=False, debug=False, num_devices=8)
    P = 128
    HO = C.HID // P
    NS = C.S // C.ST
    KVC = C.KVLR // P
    NPAIR = C.NH_G // 2
    NQT = C.S // C.QT
    NDIAG = C.QT // P
    NVS = C.S // P
    NQN = C.QFN // P
    NOT = C.HID // 512

    # ---- kernel I/O (bf16 unless noted) ----
    hT = nc.dram_tensor("hT", [C.HID, C.S], BF16, kind="ExternalInput").ap()
    w_qf = nc.dram_tensor("w_qf", [C.HID, C.QFN + C.QFR], BF16, kind="ExternalInput").ap()
    w_kva = nc.dram_tensor("w_kva", [C.HID, C.KVLR], BF16, kind="ExternalInput").ap()
    w_kbn = nc.dram_tensor("w_kbn", [C.KVLR, C.NH_G * C.DN], BF16, kind="ExternalInput").ap()
    w_kbr = nc.dram_tensor("w_kbr", [C.KVLR, C.NH_G * C.DR], BF16, kind="ExternalInput").ap()
    w_vb = nc.dram_tensor("w_vb", [C.KVLR, C.NH_G * C.DV], BF16, kind="ExternalInput").ap()
    w_ob = nc.dram_tensor("w_ob", [C.NH_G * C.DV, C.HID], BF16, kind="ExternalInput").ap()
    cos2 = nc.dram_tensor("cos2", [P, C.S], BF16, kind="ExternalInput").ap()
    ssin2 = nc.dram_tensor("ssin2", [P, C.S], BF16, kind="ExternalInput").ap()
    dmask = nc.dram_tensor("dmask", [P, P], BF16, kind="ExternalInput").ap()
    swp = nc.dram_tensor("swp", [P, P], BF16, kind="ExternalInput").ap()
    outp = nc.dram_tensor("outp", [C.S, C.HID], BF16, kind="ExternalOutput").ap()

    with tile.TileContext(nc) as tc, ExitStack() as wctx:
        # ---- weights + tables: loaded once, shared by all reps ----
        w_pool = wctx.enter_context(tc.tile_pool(name="wts", bufs=1))
        cos_sb = w_pool.tile([P, C.S], BF16)
        ssin_sb = w_pool.tile([P, C.S], BF16)
        dm_sb = w_pool.tile([P, P], BF16)
        swp_sb = w_pool.tile([P, P], BF16)
        wqf_sb = w_pool.tile([P, HO, C.QFN + C.QFR], BF16)
        wkva_sb = w_pool.tile([P, HO, C.KVLR], BF16)
        wkn_sb = w_pool.tile([P, KVC, C.NH_G * C.DN], BF16)
        wkr_sb = w_pool.tile([P, KVC, C.NH_G * C.DR], BF16)
        wv_sb = w_pool.tile([P, KVC, C.NH_G * C.DV], BF16)
        wo_sb = w_pool.tile([P, C.NH_G, C.HID], BF16)
        wqf_r = w_qf.rearrange("(ho hi) c -> hi ho c", hi=P)
        wkva_r = w_kva.rearrange("(ho hi) c -> hi ho c", hi=P)
        for ho in range(HO):
            nc.sync.dma_start(out=wqf_sb[:, ho, :], in_=wqf_r[:, ho, :])
        for ho in range(HO):
            nc.sync.dma_start(out=wkva_sb[:, ho, :], in_=wkva_r[:, ho, :])
        nc.sync.dma_start(out=cos_sb[:], in_=cos2)
        nc.sync.dma_start(out=ssin_sb[:], in_=ssin2)
        nc.sync.dma_start(out=swp_sb[:], in_=swp)
        nc.sync.dma_start(
            out=wkn_sb[:], in_=w_kbn.rearrange("(co ci) m -> ci co m", ci=P))
        nc.sync.dma_start(
            out=wkr_sb[:], in_=w_kbr.rearrange("(co ci) m -> ci co m", ci=P))
        nc.sync.dma_start(
            out=wv_sb[:], in_=w_vb.rearrange("(co ci) m -> ci co m", ci=P))
        nc.sync.dma_start(out=dm_sb[:], in_=dmask)
        nc.sync.dma_start(
            out=wo_sb[:], in_=w_ob.rearrange("(h d) o -> d h o", d=P))

        ht_pool = wctx.enter_context(tc.tile_pool(name="ht", bufs=2))
        hT_r = hT.rearrange("(ho hi) s -> hi ho s", hi=P)

        def load_ht(s0):
            # split into 4 sub-DMAs: finer dependency granularity (compute
            # starts after the first quarter) and the DMA-engine FIFO can
            # interleave weight-chunk loads.
            ht_sb = ht_pool.tile([P, HO, C.ST], BF16, tag="ht")
            for hq in range(0, HO, 4):
                nc.scalar.dma_start(
                    out=ht_sb[:, hq:hq + 4, :],
                    in_=hT_r[:, hq:hq + 4, s0:s0 + C.ST])
            return ht_sb

        # block-0 prefetch handle carried across reps: issued near the end of
        # rep r's P1 so the load lands during rep r's attention phase and
        # rep r+1's first matmul never waits on it.
        ht_next = [None]

        for rep in range(reps):
            with ExitStack() as tctx:
                # ---- persistent tiles (live across all phases) ----
                per_pool = tctx.enter_context(tc.tile_pool(name=f"persist{rep}", bufs=1))
                qTn_sb = per_pool.tile([P, NQN, C.S], BF16)
                qTr_sb = per_pool.tile([P, NPAIR, C.S], BF16)
                kTn_sb = per_pool.tile([P, C.NH_G, C.S], BF16)
                kTr_sb = per_pool.tile([P, NPAIR, C.S], BF16)
                v_sb = per_pool.tile([P, NVS, C.NH_G * C.DV], BF16)
                def rope_psum(rp_pool, rps_pool, ps_nat, dst_ap, s0, slen):
                    """psum of rope rows -> roped into dst (PE perm matmul)."""
                    tmp = rp_pool.tile([P, slen], BF16, tag="rope_tmp")
                    nc.scalar.copy(tmp[:], ps_nat[:])
                    ps2 = rps_pool.tile([P, slen], F32, tag="rope_ps2")
                    nc.tensor.matmul(ps2[:], swp_sb[:], tmp[:], start=True, stop=True)
                    m1 = rp_pool.tile([P, slen], BF16, tag="rope_m1")
                    nc.vector.tensor_mul(m1[:], tmp[:], cos_sb[:, s0:s0 + slen])
                    t2 = rp_pool.tile([P, slen], BF16, tag="rope_t2")
                    nc.vector.tensor_mul(t2[:], ps2[:], ssin_sb[:, s0:s0 + slen])
                    nc.vector.tensor_add(dst_ap, m1[:], t2[:])

                # ===== Phases 1+2 interleaved per s-block =====
                with ExitStack() as ctx:
                    rp_pool = ctx.enter_context(tc.tile_pool(name=f"rp{rep}", bufs=3))
                    kv_pool = ctx.enter_context(tc.tile_pool(name=f"kv{rep}", bufs=2))
                    ps1_pool = ctx.enter_context(
                        tc.tile_pool(name=f"ps1{rep}", bufs=3, space="PSUM"))
                    ps2_pool = ctx.enter_context(
                        tc.tile_pool(name=f"ps2{rep}", bufs=3, space="PSUM"))
                    rps_pool = ctx.enter_context(
                        tc.tile_pool(name=f"rps{rep}", bufs=2, space="PSUM"))

                    ht0_sb = ht_next[0] if ht_next[0] is not None else load_ht(0)

                    for st in range(NS):
                        s0 = st * C.ST
                        ht_sb = ht0_sb if st == 0 else load_ht(s0)

                        def accum(lhs_sb, col0):
                            ps = ps1_pool.tile([P, C.ST], F32, tag="ps1")
                            for h in range(HO):
                                nc.tensor.matmul(
                                    ps[:], lhs_sb[:, h, col0:col0 + P],
                                    ht_sb[:, h, :],
                                    start=(h == 0), stop=(h == HO - 1))
                            return ps

                        # --- phase 1: q projections + kv latent ---
                        for t in range(NQN):
                            ps = accum(wqf_sb, t * P)
                            nc.scalar.copy(qTn_sb[:, t, s0:s0 + C.ST], ps[:])
                        for pr in range(NPAIR):
                            ps_nat = accum(wqf_sb, C.QFN + pr * P)
                            rope_psum(rp_pool, rps_pool, ps_nat,
                                      qTr_sb[:, pr, s0:s0 + C.ST], s0, C.ST)
                        kv_sb = kv_pool.tile([P, KVC, C.ST], BF16, tag="kv")
                        for cc in range(KVC):
                            ps = accum(wkva_sb, cc * P)
                            nc.vector.tensor_copy(kv_sb[:, cc, :], ps[:])

                        # --- phase 2: k/v head projections from kv latent ---
                        for h in range(C.NH_G):
                            ps = ps2_pool.tile([P, C.ST], F32, tag="p2")
                            for cch in range(KVC):
                                nc.tensor.matmul(
                                    ps[:], wkn_sb[:, cch, h * C.DN:(h + 1) * C.DN],
                                    kv_sb[:, cch, :],
                                    start=(cch == 0), stop=(cch == KVC - 1))
                            nc.scalar.copy(kTn_sb[:, h, s0:s0 + C.ST], ps[:])
                        for pr in range(NPAIR):
                            ps_nat = ps2_pool.tile([P, C.ST], F32, tag="p2")
                            for cch in range(KVC):
                                nc.tensor.matmul(
                                    ps_nat[:], wkr_sb[:, cch, pr * P:(pr + 1) * P],
                                    kv_sb[:, cch, :],
                                    start=(cch == 0), stop=(cch == KVC - 1))
                            rope_psum(rp_pool, rps_pool, ps_nat,
                                      kTr_sb[:, pr, s0:s0 + C.ST], s0, C.ST)
                        for ssub in range(C.ST // P):
                            vs = (s0 + ssub * P) // P
                            ps = ps2_pool.tile([P, C.NH_G * C.DV], F32, tag="p2")
                            for cch in range(KVC):
                                nc.tensor.matmul(
                                    ps[:], kv_sb[:, cch, ssub * P:(ssub + 1) * P],
                                    wv_sb[:, cch, :],
                                    start=(cch == 0), stop=(cch == KVC - 1))
                            nc.scalar.copy(v_sb[:, vs, :], ps[:])

                ht_next[0] = load_ht(0) if rep < reps - 1 else None

                # ================= Phase 3: attention + w_o =================
                with ExitStack() as ctx:
                    e_pool = ctx.enter_context(tc.tile_pool(name=f"ae{rep}", bufs=4))
                    es_pool = ctx.enter_context(tc.tile_pool(name=f"aes{rep}", bufs=1))
                    d_pool = ctx.enter_context(tc.tile_pool(name=f"ad{rep}", bufs=1))
                    ao_pool = ctx.enter_context(tc.tile_pool(name=f"aao{rep}", bufs=2))
                    oev_pool = ctx.enter_context(tc.tile_pool(name=f"aoe{rep}", bufs=2))
                    ps_s = ctx.enter_context(
                        tc.tile_pool(name=f"apss{rep}", bufs=4, space="PSUM"))
                    ps_o = ctx.enter_context(
                        tc.tile_pool(name=f"apso{rep}", bufs=2, space="PSUM"))

                    def emit_wo(qt, ao_sb):
                        q0 = qt * C.QT
                        for qs in range(C.QT // P):
                            oev = oev_pool.tile([P, C.HID], BF16, tag="oev")
                            for ot in range(NOT):
                                psw = ps_s.tile([P, C.QT], F32, tag="pss")
                                for h in range(C.NH_G):
                                    nc.tensor.matmul(
                                        psw[:], ao_sb[:, h, qs * P:(qs + 1) * P],
                                        wo_sb[:, h, ot * 512:(ot + 1) * 512],
                                        start=(h == 0), stop=(h == C.NH_G - 1))
                                nc.scalar.copy(
                                    oev[:, ot * 512:(ot + 1) * 512], psw[:])
                            nc.sync.dma_start(
                                out=outp[q0 + qs * P:q0 + (qs + 1) * P, :],
                                in_=oev[:])

                    wo_prev = None
                    for qt in range(NQT):
                        q0 = qt * C.QT
                        nkt = (qt + 1) * C.QT // P
                        ao_sb = ao_pool.tile([P, C.NH_G, C.QT], BF16, tag="ao")
                        for pr in range(NPAIR):
                            # the two heads of a pair run interleaved through
                            # the k-loop: doubles pipeline depth so act/vector
                            # latency hides behind the PE even at small nkt.
                            esum = [es_pool.tile([P, C.QT], F32R, tag=f"es{hh}",
                                                 name=f"esum{hh}")
                                    for hh in range(2)]
                            pso = [ps_o.tile([P, C.QT], F32, tag=f"pso{hh}",
                                             name=f"pso{hh}")
                                   for hh in range(2)]

                            def consume_part(hh, e_prev, ktp, qoff):
                                nc.tensor.matmul(
                                    pso[hh][:, qoff:C.QT],
                                    v_sb[:, ktp, (pr * 2 + hh) * C.DV:
                                         (pr * 2 + hh + 1) * C.DV],
                                    e_prev[:, qoff:C.QT],
                                    start=(ktp == 0), stop=(ktp == nkt - 1))

                            pend = [[], []]
                            for kt in range(nkt):
                                k0 = kt * P
                                # diagonal k-tiles only need q >= k: shrink
                                # the moving dim to the surviving q-range.
                                j = kt - qt * NDIAG
                                qoff = max(j, 0) * P
                                for hh in range(2):
                                    h = pr * 2 + hh
                                    rsl = slice(hh * C.DR, (hh + 1) * C.DR)
                                    pss = ps_s.tile([P, C.QT], F32, tag="pss")
                                    nc.tensor.matmul(
                                        pss[:, qoff:C.QT], kTn_sb[:, h, k0:k0 + P],
                                        qTn_sb[:, h, q0 + qoff:q0 + C.QT],
                                        start=True, stop=False)
                                    nc.tensor.matmul(
                                        pss[:, qoff:C.QT], kTr_sb[rsl, pr, k0:k0 + P],
                                        qTr_sb[rsl, pr, q0 + qoff:q0 + C.QT],
                                        start=False, stop=True)
                                    e_sb = e_pool.tile([P, C.QT], BF16, tag=f"e{hh}")
                                    nc.scalar.activation(
                                        e_sb[:, qoff:C.QT], pss[:, qoff:C.QT],
                                        mybir.ActivationFunctionType.Exp,
                                        scale=C.SCALE)
                                    if j >= 0:
                                        nc.vector.tensor_mul(
                                            e_sb[:, qoff:qoff + P],
                                            e_sb[:, qoff:qoff + P], dm_sb[:])
                                    if kt == 0:
                                        nc.vector.tensor_copy(esum[hh][:], e_sb[:])
                                    else:
                                        nc.vector.tensor_add(
                                            esum[hh][:, qoff:C.QT],
                                            esum[hh][:, qoff:C.QT],
                                            e_sb[:, qoff:C.QT])
                                    # consume two iterations behind: the
                                    # V-matmul's wait on e is then satisfied
                                    # well before the PE reaches it.
                                    if len(pend[hh]) >= 2:
                                        consume_part(hh, *pend[hh].pop(0))
                                    pend[hh].append((e_sb, kt, qoff))
                            for hh in range(2):
                                h = pr * 2 + hh
                                while pend[hh]:
                                    consume_part(hh, *pend[hh].pop(0))
                                dsum = d_pool.tile([P, C.QT], F32, tag=f"ds{hh}")
                                nc.gpsimd.partition_all_reduce(
                                    dsum[:], esum[hh][:], channels=P,
                                    reduce_op=bass_isa.ReduceOp.add)
                                rec = d_pool.tile([P, C.QT], F32, tag=f"rec{hh}")
                                nc.vector.reciprocal(rec[:], dsum[:])
                                nc.vector.tensor_mul(
                                    ao_sb[:, h, :], pso[hh][:], rec[:])
                        if wo_prev is not None:
                            emit_wo(*wo_prev)
                        wo_prev = (qt, ao_sb)
                    emit_wo(*wo_prev)

    nc.compile()
    return nc


def rope_tables(C: Cfg):
    """cos2/ssin2 [128, S]: two stacked 64-row blocks (head pairs share).

    ssin carries the rotate_half sign: rows [0:32] (and [64:96]) negated.
    """
    inv = 1.0 / (10000.0 ** (np.arange(0, C.DR, 2, dtype=np.float64) / C.DR))
    freqs = np.arange(C.S, dtype=np.float64)[:, None] * inv[None, :]  # [S, 32]
    emb = np.concatenate([freqs, freqs], axis=1)  # [S, 64]
    cos = np.cos(emb).T.astype(np.float32)   # [64, S]
    sin = np.sin(emb).T.astype(np.float32)
    ssin = sin.copy()
    ssin[: C.DR // 2] = -ssin[: C.DR // 2]
    cos2 = np.concatenate([cos, cos], axis=0)     # [128, S]
    ssin2 = np.concatenate([ssin, ssin], axis=0)
    return np.ascontiguousarray(cos2), np.ascontiguousarray(ssin2)


def host_inputs(C: Cfg, inputs: dict, core: int):
    """Build the per-core input map from full inputs (cast to bf16)."""
    import ml_dtypes

    bf16 = ml_dtypes.bfloat16
    NH = inputs["w_q_nope"].shape[1] // C.DN
    groups = NH // C.NH_G
    b = core // groups
    g = core % groups
    hs = slice(g * C.NH_G, (g + 1) * C.NH_G)

    cast = lambda x: np.ascontiguousarray(np.asarray(x, dtype=np.float32).astype(bf16))

    hT = cast(inputs["hidden_states"][b].T)
    w_q_a = np.asarray(inputs["w_q_a"], dtype=np.float32)
    w_qbn = np.asarray(inputs["w_q_nope"], dtype=np.float32).reshape(
        C.QLR, NH, C.DN)[:, hs].reshape(C.QLR, -1)
    w_qbr = np.asarray(inputs["w_q_rope"], dtype=np.float32).reshape(
        C.QLR, NH, C.DR)[:, hs].reshape(C.QLR, -1)
    w_qf = cast(np.concatenate([w_q_a @ w_qbn, w_q_a @ w_qbr], axis=1))
    w_kva = cast(inputs["w_kv_a"])
    w_kbn = cast(np.asarray(inputs["w_k_nope"], dtype=np.float32).reshape(
        C.KVLR, NH, C.DN)[:, hs].reshape(C.KVLR, -1))
    w_kbr = cast(np.asarray(inputs["w_k_rope"], dtype=np.float32).reshape(
        C.KVLR, NH, C.DR)[:, hs].reshape(C.KVLR, -1))
    w_vb = cast(np.asarray(inputs["w_v"], dtype=np.float32).reshape(
        C.KVLR, NH, C.DV)[:, hs].reshape(C.KVLR, -1))
    w_ob = cast(np.asarray(inputs["w_o"], dtype=np.float32).reshape(
        NH, C.DV, C.HID)[hs].reshape(-1, C.HID))
    cos2, ssin2 = rope_tables(C)
    # [k, q] triangle for one 128x128 diagonal sub-block: keep q >= k
    dmask = np.triu(np.ones((128, 128), dtype=np.float32))
    swp = np.zeros((128, 128), dtype=np.float32)
    for k in range(128):
        swp[k, k ^ 32] = 1.0
    return {
        "hT": hT, "w_qf": w_qf, "w_kva": w_kva,
        "w_kbn": w_kbn, "w_kbr": w_kbr, "w_vb": w_vb, "w_ob": w_ob,
        "cos2": cast(cos2), "ssin2": cast(ssin2),
        "dmask": cast(dmask), "swp": cast(swp),
    }


_NC_CACHE = {}


def kernel(**inputs) -> np.ndarray:
    from concourse.bass_utils import run_bass_kernel_spmd

    C = CFG
    if "nc" not in _NC_CACHE:
        _NC_CACHE["nc"] = build_nc(C)
    nc = _NC_CACHE["nc"]

    in_maps = [host_inputs(C, inputs, c) for c in range(8)]
    res = run_bass_kernel_spmd(nc, in_maps, core_ids=list(range(8)))

    B = inputs["hidden_states"].shape[0]
    groups = 8 // B
    out = np.zeros((B, C.S, C.HID), dtype=np.float32)
    for c in range(8):
        out[c // groups] += np.asarray(res.results[c]["outp"], dtype=np.float32)
    return out
